# revision 1
# baseline (speedup 1.0000x reference)
"""Multi-head attention (b=4, n=2048, dim=1024, heads=16, hd=64) on 8 TRN2
NeuronCores.

Sharding: core i = (batch b = i//2, query-half h = i%2). Fully local — each
core recomputes K/V for its batch's full 2048 tokens (+25% FLOPs, zero
communication), computes Q for its 1024 tokens, per-head attention with
transposed scores S^T[k, q] (Q/K stay feature-major straight from the QKV
matmuls), softmax without max-subtraction (scores ~N(0, 0.33^2)), row sums via
an appended ones-column on V, then the out-projection. All matmul inputs bf16
(PSUM f32). The host does all sharding / transposes / bias folds in numpy and
reassembles the output.

Layouts on device (feature-major, partition dim first):
  xT   [128, 8 dc, 2048 t]   x^T, d-chunked; local-half tokens first
  qT   [128, 8 fc, 1024 t]   Q^T = wqT.T @ xT[:, :1024] + bq
  kT   [128, 8 fc, 2048 t]   K^T
  v    [128, 16 tt, 16 h, 65] V token-major per head, col 64 == 1.0 (sum row)
  S^T  psum [128 k, 512 q] = kT_h_slice.T @ qT_h_slice   (contraction hd=64)
  P~   exp(S^T/8) bf16; PV: psum_o[65, 512] += v_aug.T @ P~ (row 64 = sums)
  attn [128, 8 fc, 1024 t]   normalized, head-concat feature-major
  out  [1024 e, 1024 t]^T -> DMA'd as outT; host transposes back
"""
import sys

sys.path.insert(0, "/opt/trn_rl_repo")

import numpy as np
import ml_dtypes

import concourse.bass as bass
import concourse.tile as tile
from concourse import bacc, mybir
from concourse.bass_utils import run_bass_kernel_spmd

BF16 = mybir.dt.bfloat16
F32 = mybir.dt.float32
EXP = mybir.ActivationFunctionType.Exp
MULT = mybir.AluOpType.mult

D = 1024          # model dim
DC = 8            # d chunks of 128
NT = 2048         # kv tokens per core
NQ = 1024         # q tokens per core
NH = 16           # heads
HD = 64           # head dim
QC = 512          # q chunk (psum free)
NKT = 16          # k tiles of 128
N_CORES = 8

_CACHE = {}


def _install_ntff_shim():
    """The agent image's ``antenv`` lacks ``axon_hooks``, so concourse's
    trace=True path can't find the NTFF profile hook even though
    ``libaxon_pjrt.so`` supports it. Recreate the glue (same contract as
    trn_boot's ``_ntff_profile_via_ctypes``)."""
    import types
    import ctypes
    import contextlib

    if "antenv.axon_hooks" in sys.modules:
        return
    so_path = "/opt/axon/libaxon_pjrt.so"
    try:
        lib = ctypes.CDLL(so_path)
        if not hasattr(lib, "axon_start_nrt_profile"):
            return
    except OSError:
        return
    lib.axon_start_nrt_profile.argtypes = [ctypes.POINTER(ctypes.c_int64),
                                           ctypes.c_size_t]
    lib.axon_start_nrt_profile.restype = ctypes.c_int64
    lib.axon_stop_nrt_profile.argtypes = [ctypes.c_char_p]
    lib.axon_stop_nrt_profile.restype = ctypes.c_int64

    @contextlib.contextmanager
    def _hook(output_dir, device_ids):
        import jax
        jax.devices()
        if device_ids:
            ids = (ctypes.c_int64 * len(device_ids))(*device_ids)
            rc = lib.axon_start_nrt_profile(ids, len(device_ids))
        else:
            rc = lib.axon_start_nrt_profile(None, 0)
        if rc != 0:
            raise RuntimeError(f"axon_start_nrt_profile rc={rc}")
        try:
            yield
        finally:
            n = lib.axon_stop_nrt_profile(str(output_dir).encode())
            print(f"ntff profile: {n} file(s) written to {output_dir}",
                  file=sys.stderr)

    mod = types.ModuleType("antenv.axon_hooks")
    _h = [_hook]
    mod.set_axon_ntff_profile_hook = lambda h: _h.__setitem__(0, h)
    mod.get_axon_ntff_profile_hook = lambda: _h[0]
    sys.modules["antenv.axon_hooks"] = mod
    import antenv
    antenv.axon_hooks = mod


def build():
    nc = bacc.Bacc("TRN2", target_bir_lowering=False, debug=False,
                   num_devices=N_CORES)

    xT_d = nc.dram_tensor("xT", [D, NT], BF16, kind="ExternalInput")
    wq_d = nc.dram_tensor("wqT", [D, D], BF16, kind="ExternalInput")
    wk_d = nc.dram_tensor("wkT", [D, D], BF16, kind="ExternalInput")
    wv_d = nc.dram_tensor("wvT", [D, D], BF16, kind="ExternalInput")
    ow_d = nc.dram_tensor("owT", [D, D], BF16, kind="ExternalInput")
    bq_d = nc.dram_tensor("bq", [128, DC], F32, kind="ExternalInput")
    bk_d = nc.dram_tensor("bk", [128, DC], F32, kind="ExternalInput")
    ob_d = nc.dram_tensor("ob", [128, DC], F32, kind="ExternalInput")
    out_d = nc.dram_tensor("outT", [D, NQ], F32, kind="ExternalOutput")

    chunked = lambda t: t.ap().rearrange("(c p) t -> p c t", p=128)

    with tile.TileContext(nc) as tc:
        # ---------- persistent SBUF ----------
        with tc.tile_pool(name="persist", bufs=1) as persist:
            kT = persist.tile([128, DC, NT], BF16)
            qT = persist.tile([128, DC, NQ], BF16)
            v = persist.tile([128, NKT, NH, HD + 1], BF16)
            attn = persist.tile([128, DC, NQ], BF16)
            bq_sb = persist.tile([128, DC], F32)
            bk_sb = persist.tile([128, DC], F32)
            ob_sb = persist.tile([128, DC], F32)
            # ones columns for the PV sum row (v-proj epilogue writes skip col 64)
            nc.vector.memset(v, 1.0)
            warm = persist.tile([128, 1], F32)
            nc.vector.memset(warm, 0.0)

            # One static PSUM budget for the whole kernel (8 banks):
            #   ps_acc 3x[128,512] (proj + out-proj accumulators)  = 3
            #   ps_s   2x[128,2,512] (attention scores, 2 k-tiles) = 4
            #   ps_o   1x[65,512]   (PV accumulator)               = 1
            # (the PV accumulator is evacuated by a fast DVE copy ~2us after
            # the last PV matmul, and the next head's PV is ~18us away, so a
            # single buffer suffices; the third acc buffer absorbs the
            # proj-matmul-waits-on-DVE-evacuation stalls seen in traces)
            SB = 2  # k-tiles per score batch
            with tc.tile_pool(name="w1", bufs=1) as w1, \
                 tc.tile_pool(name="xpool", bufs=1) as xpool, \
                 tc.tile_pool(name="ppool", bufs=3) as ppool, \
                 tc.tile_pool(name="nrm", bufs=2) as nrm, \
                 tc.tile_pool(name="fout", bufs=3) as fout, \
                 tc.tile_pool(name="drpool", bufs=4, space="DRAM") as drpool, \
                 tc.tile_pool(name="ps_acc", bufs=3, space="PSUM") as ps_acc, \
                 tc.tile_pool(name="ps_s", bufs=2, space="PSUM") as ps_s, \
                 tc.tile_pool(name="ps_o", bufs=1, space="PSUM") as ps_o:
                xT = xpool.tile([128, DC, NT], BF16)
                wq = w1.tile([128, DC, D], BF16, tag="wq")
                wk = w1.tile([128, DC, D], BF16, tag="wk")
                wv = w1.tile([128, DC, D], BF16, tag="wv")
                # ow shares wq's slot: loaded after Q-proj finishes with wq
                ow = w1.tile([128, DC, D], BF16, tag="wq", name="ow")
                # per-chunk loads so the first projection matmuls can start
                # as soon as chunk 0 lands; spread across three idle HWDGE
                # queues (the first K-proj chain needs all 8 chunks of
                # wk+xT, ~6MB, before its accumulation can finish)
                for dc in range(DC):
                    nc.scalar.dma_start(out=wk[:, dc, :],
                                        in_=chunked(wk_d)[:, dc, :])
                    nc.sync.dma_start(out=xT[:, dc, :],
                                      in_=chunked(xT_d)[:, dc, :])
                # biases after the first-matmul-critical chunk loads, before
                # the big weight transfers (first K epilogue needs bk ~25us)
                nc.sync.dma_start(out=bq_sb, in_=bq_d.ap())
                nc.sync.dma_start(out=bk_sb, in_=bk_d.ap())
                nc.sync.dma_start(out=ob_sb, in_=ob_d.ap())
                nc.sync.dma_start(out=wq, in_=chunked(wq_d))
                nc.sync.dma_start(out=wv, in_=chunked(wv_d))
                nc.sync.dma_start(out=ow, in_=chunked(ow_d))
                # dummy exp pulls the ~2.7us ACT_TABLE_LOAD off the first
                # real score tile's critical path; emitted AFTER the weight
                # DMA issues so it doesn't delay them on the ACT queue
                nc.scalar.activation(warm, warm, EXP)

                def proj_kq(fc):
                    # K^T / Q^T feature-chunk fc (feature-major)
                    for t4 in range(4):
                        ps = ps_acc.tile([128, QC], F32, tag="ps")
                        for dc in range(DC):
                            nc.tensor.matmul(
                                ps,
                                lhsT=wk[:, dc, fc * 128:(fc + 1) * 128],
                                rhs=xT[:, dc, t4 * QC:(t4 + 1) * QC],
                                start=(dc == 0), stop=(dc == DC - 1))
                        nc.vector.tensor_scalar_add(
                            kT[:, fc, t4 * QC:(t4 + 1) * QC], ps,
                            bk_sb[:, fc:fc + 1])
                    for t2 in range(2):
                        ps = ps_acc.tile([128, QC], F32, tag="ps")
                        for dc in range(DC):
                            nc.tensor.matmul(
                                ps,
                                lhsT=wq[:, dc, fc * 128:(fc + 1) * 128],
                                rhs=xT[:, dc, t2 * QC:(t2 + 1) * QC],
                                start=(dc == 0), stop=(dc == DC - 1))
                        nc.vector.tensor_scalar_add(
                            qT[:, fc, t2 * QC:(t2 + 1) * QC], ps,
                            bq_sb[:, fc:fc + 1])

                def proj_v(f2):
                    # V token-major, feature half f2 (heads 8*f2 .. 8*f2+7);
                    # epilogue into [.., 65]-strided per-head slots (ones col
                    # survives from the memset; bv folded into ob on host)
                    for tt in range(NKT):
                        ps = ps_acc.tile([128, QC], F32, tag="ps")
                        for dc in range(DC):
                            nc.tensor.matmul(
                                ps,
                                lhsT=xT[:, dc, tt * 128:(tt + 1) * 128],
                                rhs=wv[:, dc, f2 * QC:(f2 + 1) * QC],
                                start=(dc == 0), stop=(dc == DC - 1))
                        nc.vector.tensor_copy(
                            out=v[:, tt, f2 * 8:(f2 + 1) * 8, 0:HD],
                            in_=ps.rearrange("p (h d) -> p h d", d=HD))

                def attn_head(h, qc):
                    fc, hi = h // 2, (h % 2) * 64
                    qsl = slice(qc * QC, (qc + 1) * QC)
                    po = ps_o.tile([HD + 1, QC], F32, tag="po")
                    pts = []
                    for b in range(NKT // SB):
                        ss = ps_s.tile([128, SB, QC], F32, tag="ss")
                        for j in range(SB):
                            kt = b * SB + j
                            nc.tensor.matmul(
                                ss[:, j, :],
                                lhsT=kT[hi:hi + HD, fc,
                                        kt * 128:(kt + 1) * 128],
                                rhs=qT[hi:hi + HD, fc, qsl],
                                start=True, stop=True)
                        pt = ppool.tile([128, SB, QC], BF16, tag="pt")
                        nc.scalar.activation(pt, ss, EXP, scale=0.125)
                        pts.append(pt)
                    for b, pt in enumerate(pts):
                        for j in range(SB):
                            ki = b * SB + j
                            nc.tensor.matmul(
                                po,
                                lhsT=v[:, ki, h, :],
                                rhs=pt[:, j, :],
                                start=(ki == 0), stop=(ki == NKT - 1))
                    # Evacuate the PV accumulator to SBUF with one fast copy
                    # (frees the PSUM bank), then normalize from SBUF:
                    # 1/sum row, DRAM-bounce partition broadcast, multiply.
                    # bufs=3: with ps_o single-buffered, this evacuation
                    # copy gates the next head's PV — extra slack here keeps
                    # the DVE chain off that critical edge
                    ps_sb = nrm.tile([HD + 1, QC], F32, tag="ps_sb", bufs=3)
                    nc.vector.tensor_copy(out=ps_sb, in_=po)
                    rc = nrm.tile([128, QC], F32, tag="rc")
                    nc.vector.reciprocal(rc[HD:HD + 1, :],
                                         ps_sb[HD:HD + 1, :])
                    dr = drpool.tile([1, QC], F32, tag="dr")
                    nc.sync.dma_start(out=dr, in_=rc[HD:HD + 1, :])
                    bc = nrm.tile([64, QC], F32, tag="bc")
                    nc.sync.dma_start(
                        out=bc,
                        in_=bass.AP(tensor=dr.tensor, offset=dr.offset,
                                    ap=[[0, 64], dr.ap[1]]))
                    if hi == 0:
                        nc.vector.tensor_tensor(
                            out=attn[0:HD, fc, qsl],
                            in0=ps_sb[0:HD, :], in1=bc, op=MULT)
                    else:
                        sh = nrm.tile([64, QC], BF16, tag="sh")
                        nc.vector.tensor_tensor(
                            out=sh, in0=ps_sb[0:HD, :], in1=bc, op=MULT)
                        nc.sync.dma_start(out=attn[64:128, fc, qsl],
                                          in_=sh)

                def out_proj(ec, t2):
                    ps = ps_acc.tile([128, QC], F32, tag="ps")
                    for fc in range(DC):
                        nc.tensor.matmul(
                            ps,
                            lhsT=ow[:, fc, ec * 128:(ec + 1) * 128],
                            rhs=attn[:, fc, t2 * QC:(t2 + 1) * QC],
                            start=(fc == 0), stop=(fc == DC - 1))
                    fo = fout.tile([128, QC], F32, tag="fo")
                    # bias-add on ACT: it is idle after its last exp, exactly
                    # when DVE is the tail bottleneck
                    nc.scalar.activation(fo, ps,
                                         mybir.ActivationFunctionType.Identity,
                                         bias=ob_sb[:, ec:ec + 1])
                    nc.sync.dma_start(
                        out=out_d.ap()[ec * 128:(ec + 1) * 128,
                                       t2 * QC:(t2 + 1) * QC],
                        in_=fo)

                # Interleaved emission: attention for head pair (2fc-2, 2fc-1)
                # right after K/Q chunk fc lands, V halves as needed.
                # Interleaved emission as before, but the last two pairs run
                # qc=0 before any of their qc=1 work, so out-proj t2=0
                # becomes PE filler during the ACT-bound endgame (otherwise
                # the PE duty cycle collapses there, HAM halves the clock,
                # and the tail runs cold).
                proj_kq(0)
                proj_v(0)
                proj_kq(1)
                for fc in range(2, DC):
                    for h in (2 * fc - 4, 2 * fc - 3):
                        for qc in range(2):
                            attn_head(h, qc)
                    if fc == 4:
                        proj_v(1)
                    proj_kq(fc)
                for h in (NH - 4, NH - 3):          # pair 6, both qc
                    for qc in range(2):
                        attn_head(h, qc)
                for h in (NH - 2, NH - 1):          # pair 7 qc0
                    attn_head(h, 0)
                # pair 7 qc1 emitted BEFORE out-proj t2=0: out-proj's fc0-6
                # matmuls are dependency-ready anyway (all qc0 heads done)
                # and fill endgame PE gaps, but this queue order keeps the
                # last odd-head shift DMA ahead of out-proj's output DMAs
                # on the sync queue
                for h in (NH - 2, NH - 1):
                    attn_head(h, 1)
                for ec in range(DC):
                    out_proj(ec, 0)
                for ec in range(DC):
                    out_proj(ec, 1)

    nc.compile()
    return nc


def _prep_in_maps(x, qkv_w, qkv_b, out_w, out_b):
    bf = ml_dtypes.bfloat16
    wqT = np.ascontiguousarray(qkv_w[0:D].T).astype(bf)
    wkT = np.ascontiguousarray(qkv_w[D:2 * D].T).astype(bf)
    wvT = np.ascontiguousarray(qkv_w[2 * D:3 * D].T).astype(bf)
    owT = np.ascontiguousarray(out_w.T).astype(bf)
    bq = np.ascontiguousarray(qkv_b[0:D].reshape(DC, 128).T).astype(np.float32)
    bk = np.ascontiguousarray(qkv_b[D:2 * D].reshape(DC, 128).T).astype(np.float32)
    ob_eff = out_b + out_w @ qkv_b[2 * D:3 * D]
    ob = np.ascontiguousarray(ob_eff.reshape(DC, 128).T).astype(np.float32)

    in_maps = []
    for i in range(N_CORES):
        b, h = i // 2, i % 2
        xb = x[b]
        xp = np.concatenate([xb[h * NQ:(h + 1) * NQ],
                             xb[(1 - h) * NQ:(2 - h) * NQ]], 0)
        xT = np.ascontiguousarray(xp.T).astype(bf)
        in_maps.append(dict(xT=xT, wqT=wqT, wkT=wkT, wvT=wvT, owT=owT,
                            bq=bq, bk=bk, ob=ob))
    return in_maps


def run(x, qkv_w, qkv_b, out_w, out_b, trace=False):
    if trace:
        _install_ntff_shim()
    if "nc" not in _CACHE:
        _CACHE["nc"] = build()
    nc = _CACHE["nc"]
    in_maps = _prep_in_maps(np.asarray(x, np.float32),
                            np.asarray(qkv_w, np.float32),
                            np.asarray(qkv_b, np.float32),
                            np.asarray(out_w, np.float32),
                            np.asarray(out_b, np.float32))
    res = run_bass_kernel_spmd(nc, in_maps, core_ids=list(range(N_CORES)),
                               trace=trace)
    out = np.empty((4, 2048, D), np.float32)
    for i in range(N_CORES):
        b, h = i // 2, i % 2
        out[b, h * NQ:(h + 1) * NQ] = res.results[i]["outT"].T
    return out, res


def kernel(**inputs):
    out, _ = run(**inputs)
    return out



# revision 6
# speedup vs baseline: 1.1565x; 1.1565x over previous
"""Multi-head attention (b=4, n=2048, dim=1024, heads=16, hd=64) on 8 TRN2
NeuronCores.

Sharding: core i = (batch b = i//2, query-half h = i%2). Fully local — each
core recomputes K/V for its batch's full 2048 tokens (+25% FLOPs, zero
communication), computes Q for its 1024 tokens, per-head attention with
transposed scores S^T[k, q] (Q/K stay feature-major straight from the QKV
matmuls), softmax without max-subtraction (scores ~N(0, 0.33^2)), row sums via
an appended ones-column on V, then the out-projection. All matmul inputs bf16
(PSUM f32). The host does all sharding / transposes / bias folds in numpy and
reassembles the output.

Layouts on device (feature-major, partition dim first):
  xT   [128, 8 dc, 2048 t]   x^T, d-chunked; local-half tokens first
  qT   [128, 8 fc, 1024 t]   Q^T = wqT.T @ xT[:, :1024] + bq
  kT   [128, 8 fc, 2048 t]   K^T
  v    [128, 16 tt, 16 h, 65] V token-major per head, col 64 == 1.0 (sum row)
  S^T  psum [128 k, 512 q] = kT_h_slice.T @ qT_h_slice   (contraction hd=64)
  P~   exp(S^T/8) bf16; PV: psum_o[65, 512] += v_aug.T @ P~ (row 64 = sums)
  attn [128, 8 fc, 1024 t]   normalized, head-concat feature-major
  out  [1024 e, 1024 t]^T -> DMA'd as outT; host transposes back
"""
import sys

sys.path.insert(0, "/opt/trn_rl_repo")

import numpy as np
import ml_dtypes

import concourse.bass as bass
import concourse.tile as tile
from concourse import bacc, mybir
from concourse.bass_utils import run_bass_kernel_spmd

BF16 = mybir.dt.bfloat16
F32 = mybir.dt.float32
EXP = mybir.ActivationFunctionType.Exp
MULT = mybir.AluOpType.mult

D = 1024          # model dim
DC = 8            # d chunks of 128
NT = 2048         # kv tokens per core
NQ = 1024         # q tokens per core
NH = 16           # heads
HD = 64           # head dim
QC = 512          # q chunk (psum free)
NKT = 16          # k tiles of 128
N_CORES = 8

_CACHE = {}


def _install_ntff_shim():
    """The agent image's ``antenv`` lacks ``axon_hooks``, so concourse's
    trace=True path can't find the NTFF profile hook even though
    ``libaxon_pjrt.so`` supports it. Recreate the glue (same contract as
    trn_boot's ``_ntff_profile_via_ctypes``)."""
    import types
    import ctypes
    import contextlib

    if "antenv.axon_hooks" in sys.modules:
        return
    so_path = "/opt/axon/libaxon_pjrt.so"
    try:
        lib = ctypes.CDLL(so_path)
        if not hasattr(lib, "axon_start_nrt_profile"):
            return
    except OSError:
        return
    lib.axon_start_nrt_profile.argtypes = [ctypes.POINTER(ctypes.c_int64),
                                           ctypes.c_size_t]
    lib.axon_start_nrt_profile.restype = ctypes.c_int64
    lib.axon_stop_nrt_profile.argtypes = [ctypes.c_char_p]
    lib.axon_stop_nrt_profile.restype = ctypes.c_int64

    @contextlib.contextmanager
    def _hook(output_dir, device_ids):
        import jax
        jax.devices()
        if device_ids:
            ids = (ctypes.c_int64 * len(device_ids))(*device_ids)
            rc = lib.axon_start_nrt_profile(ids, len(device_ids))
        else:
            rc = lib.axon_start_nrt_profile(None, 0)
        if rc != 0:
            raise RuntimeError(f"axon_start_nrt_profile rc={rc}")
        try:
            yield
        finally:
            n = lib.axon_stop_nrt_profile(str(output_dir).encode())
            print(f"ntff profile: {n} file(s) written to {output_dir}",
                  file=sys.stderr)

    mod = types.ModuleType("antenv.axon_hooks")
    _h = [_hook]
    mod.set_axon_ntff_profile_hook = lambda h: _h.__setitem__(0, h)
    mod.get_axon_ntff_profile_hook = lambda: _h[0]
    sys.modules["antenv.axon_hooks"] = mod
    import antenv
    antenv.axon_hooks = mod


def build():
    nc = bacc.Bacc("TRN2", target_bir_lowering=False, debug=False,
                   num_devices=N_CORES)

    xT_d = nc.dram_tensor("xT", [D, NT], BF16, kind="ExternalInput")
    wq_d = nc.dram_tensor("wqT", [D, D], BF16, kind="ExternalInput")
    wk_d = nc.dram_tensor("wkT", [D, D], BF16, kind="ExternalInput")
    wv_d = nc.dram_tensor("wvT", [D, D], BF16, kind="ExternalInput")
    ow_d = nc.dram_tensor("owT", [D, D], BF16, kind="ExternalInput")
    bq_d = nc.dram_tensor("bq", [128, DC], F32, kind="ExternalInput")
    bk_d = nc.dram_tensor("bk", [128, DC], F32, kind="ExternalInput")
    ob_d = nc.dram_tensor("ob", [128, DC], F32, kind="ExternalInput")
    out_d = nc.dram_tensor("outT", [D, NQ], F32, kind="ExternalOutput")

    chunked = lambda t: t.ap().rearrange("(c p) t -> p c t", p=128)

    with tile.TileContext(nc) as tc:
        # ---------- persistent SBUF ----------
        with tc.tile_pool(name="persist", bufs=1) as persist:
            kT = persist.tile([128, DC, NT], BF16)
            qT = persist.tile([128, DC, NQ], BF16)
            v = persist.tile([128, NKT, NH, HD + 1], BF16)
            attn = persist.tile([128, DC, NQ], BF16)
            bq_sb = persist.tile([128, DC], F32)
            bk_sb = persist.tile([128, DC], F32)
            ob_sb = persist.tile([128, DC], F32)
            # ones columns for the PV sum row (v-proj epilogue writes skip col 64)
            nc.vector.memset(v, 1.0)
            warm = persist.tile([128, 1], F32)
            nc.vector.memset(warm, 0.0)

            # One static PSUM budget for the whole kernel (8 banks):
            #   ps_acc 2x[128,512] (proj + out-proj accumulators)    = 2
            #   ps_s   2x[128,2,512] (scores, even+odd head per kt)  = 4
            #   ps_o   2x[65,512]   (PV accumulators, even+odd head) = 2
            SB = 2  # heads per score batch (even/odd of a pair)
            with tc.tile_pool(name="w1", bufs=1) as w1, \
                 tc.tile_pool(name="xpool", bufs=1) as xpool, \
                 tc.tile_pool(name="ppool", bufs=3) as ppool, \
                 tc.tile_pool(name="nrm", bufs=2) as nrm, \
                 tc.tile_pool(name="fout", bufs=3) as fout, \
                 tc.tile_pool(name="drpool", bufs=4, space="DRAM") as drpool, \
                 tc.tile_pool(name="ps_acc", bufs=2, space="PSUM") as ps_acc, \
                 tc.tile_pool(name="ps_s", bufs=2, space="PSUM") as ps_s, \
                 tc.tile_pool(name="ps_o", bufs=2, space="PSUM") as ps_o:
                xT = xpool.tile([128, DC, NT], BF16)
                wq = w1.tile([128, DC, D], BF16, tag="wq")
                wk = w1.tile([128, DC, D], BF16, tag="wk")
                wv = w1.tile([128, DC, D], BF16, tag="wv")
                # ow shares wq's slot: loaded after Q-proj finishes with wq
                ow = w1.tile([128, DC, D], BF16, tag="wq", name="ow")
                # per-chunk loads so the first projection matmuls can start
                # as soon as chunk 0 lands; spread across three idle HWDGE
                # queues (the first K-proj chain needs all 8 chunks of
                # wk+xT, ~6MB, before its accumulation can finish)
                for dc in range(DC):
                    nc.scalar.dma_start(out=wk[:, dc, :],
                                        in_=chunked(wk_d)[:, dc, :])
                    nc.sync.dma_start(out=xT[:, dc, :],
                                      in_=chunked(xT_d)[:, dc, :])
                # biases after the first-matmul-critical chunk loads, before
                # the big weight transfers (first K epilogue needs bk ~25us)
                nc.sync.dma_start(out=bq_sb, in_=bq_d.ap())
                nc.sync.dma_start(out=bk_sb, in_=bk_d.ap())
                nc.sync.dma_start(out=ob_sb, in_=ob_d.ap())
                nc.sync.dma_start(out=wq, in_=chunked(wq_d))
                nc.sync.dma_start(out=wv, in_=chunked(wv_d))
                nc.sync.dma_start(out=ow, in_=chunked(ow_d))
                # dummy exp pulls the ~2.7us ACT_TABLE_LOAD off the first
                # real score tile's critical path; emitted AFTER the weight
                # DMA issues so it doesn't delay them on the ACT queue
                nc.scalar.activation(warm, warm, EXP)

                def proj_kq(fc):
                    # K^T / Q^T feature-chunk fc (feature-major)
                    for t4 in range(4):
                        ps = ps_acc.tile([128, QC], F32, tag="ps")
                        for dc in range(DC):
                            nc.tensor.matmul(
                                ps,
                                lhsT=wk[:, dc, fc * 128:(fc + 1) * 128],
                                rhs=xT[:, dc, t4 * QC:(t4 + 1) * QC],
                                start=(dc == 0), stop=(dc == DC - 1))
                        nc.vector.tensor_scalar_add(
                            kT[:, fc, t4 * QC:(t4 + 1) * QC], ps,
                            bk_sb[:, fc:fc + 1])
                    for t2 in range(2):
                        ps = ps_acc.tile([128, QC], F32, tag="ps")
                        for dc in range(DC):
                            nc.tensor.matmul(
                                ps,
                                lhsT=wq[:, dc, fc * 128:(fc + 1) * 128],
                                rhs=xT[:, dc, t2 * QC:(t2 + 1) * QC],
                                start=(dc == 0), stop=(dc == DC - 1))
                        nc.vector.tensor_scalar_add(
                            qT[:, fc, t2 * QC:(t2 + 1) * QC], ps,
                            bq_sb[:, fc:fc + 1])

                def proj_v(f2):
                    # V token-major, feature half f2 (heads 8*f2 .. 8*f2+7);
                    # epilogue into [.., 65]-strided per-head slots (ones col
                    # survives from the memset; bv folded into ob on host)
                    for tt in range(NKT):
                        ps = ps_acc.tile([128, QC], F32, tag="ps")
                        for dc in range(DC):
                            nc.tensor.matmul(
                                ps,
                                lhsT=xT[:, dc, tt * 128:(tt + 1) * 128],
                                rhs=wv[:, dc, f2 * QC:(f2 + 1) * QC],
                                start=(dc == 0), stop=(dc == DC - 1))
                        nc.vector.tensor_copy(
                            out=v[:, tt, f2 * 8:(f2 + 1) * 8, 0:HD],
                            in_=ps.rearrange("p (h d) -> p h d", d=HD))

                def attn_pair(fc, qc):
                    # Both heads of feature-chunk fc: even head at partitions
                    # 0:64, odd at 64:128.  Emitting the two score matmuls
                    # back-to-back puts them on disjoint PE row groups
                    # (tile_position (0,0) / (64,0)) so the HW runs them
                    # concurrently (~2x score throughput); their LDWEIGHTS
                    # pull ahead across row groups as well.
                    qsl = slice(qc * QC, (qc + 1) * QC)
                    po_e = ps_o.tile([HD + 1, QC], F32, tag="po")
                    po_o = ps_o.tile([HD + 1, QC], F32, tag="po")
                    for kt in range(NKT):
                        ss = ps_s.tile([128, SB, QC], F32, tag="ss")
                        for j in range(SB):
                            hi = j * 64
                            nc.tensor.matmul(
                                ss[:, j, :],
                                lhsT=kT[hi:hi + HD, fc,
                                        kt * 128:(kt + 1) * 128],
                                rhs=qT[hi:hi + HD, fc, qsl],
                                start=True, stop=True)
                        pt = ppool.tile([128, SB, QC], BF16, tag="pt")
                        nc.scalar.activation(pt, ss, EXP, scale=0.125)
                        nc.tensor.matmul(
                            po_e,
                            lhsT=v[:, kt, 2 * fc, :],
                            rhs=pt[:, 0, :],
                            start=(kt == 0), stop=(kt == NKT - 1))
                        nc.tensor.matmul(
                            po_o,
                            lhsT=v[:, kt, 2 * fc + 1, :],
                            rhs=pt[:, 1, :],
                            start=(kt == 0), stop=(kt == NKT - 1))
                    # Evacuate each PV accumulator to SBUF with one fast copy
                    # (frees the PSUM bank), then normalize from SBUF:
                    # 1/sum row, DRAM-bounce partition broadcast, multiply.
                    for hi, po in ((0, po_e), (64, po_o)):
                        ps_sb = nrm.tile([HD + 1, QC], F32, tag="ps_sb",
                                         bufs=3)
                        nc.vector.tensor_copy(out=ps_sb, in_=po)
                        rc = nrm.tile([128, QC], F32, tag="rc")
                        nc.vector.reciprocal(rc[HD:HD + 1, :],
                                             ps_sb[HD:HD + 1, :])
                        dr = drpool.tile([1, QC], F32, tag="dr")
                        nc.sync.dma_start(out=dr, in_=rc[HD:HD + 1, :])
                        bc = nrm.tile([64, QC], F32, tag="bc")
                        nc.sync.dma_start(
                            out=bc,
                            in_=bass.AP(tensor=dr.tensor, offset=dr.offset,
                                        ap=[[0, 64], dr.ap[1]]))
                        if hi == 0:
                            nc.vector.tensor_tensor(
                                out=attn[0:HD, fc, qsl],
                                in0=ps_sb[0:HD, :], in1=bc, op=MULT)
                        else:
                            sh = nrm.tile([64, QC], BF16, tag="sh")
                            nc.vector.tensor_tensor(
                                out=sh, in0=ps_sb[0:HD, :], in1=bc, op=MULT)
                            nc.sync.dma_start(out=attn[64:128, fc, qsl],
                                              in_=sh)

                def out_proj(ec, t2):
                    ps = ps_acc.tile([128, QC], F32, tag="ps")
                    for fc in range(DC):
                        nc.tensor.matmul(
                            ps,
                            lhsT=ow[:, fc, ec * 128:(ec + 1) * 128],
                            rhs=attn[:, fc, t2 * QC:(t2 + 1) * QC],
                            start=(fc == 0), stop=(fc == DC - 1))
                    fo = fout.tile([128, QC], F32, tag="fo")
                    # bias-add on ACT: it is idle after its last exp, exactly
                    # when DVE is the tail bottleneck
                    nc.scalar.activation(fo, ps,
                                         mybir.ActivationFunctionType.Identity,
                                         bias=ob_sb[:, ec:ec + 1])
                    nc.sync.dma_start(
                        out=out_d.ap()[ec * 128:(ec + 1) * 128,
                                       t2 * QC:(t2 + 1) * QC],
                        in_=fo)

                # Interleaved emission: attention for head pair (2fc-2, 2fc-1)
                # right after K/Q chunk fc lands, V halves as needed.
                # Interleaved emission as before, but the last two pairs run
                # qc=0 before any of their qc=1 work, so out-proj t2=0
                # becomes PE filler during the ACT-bound endgame (otherwise
                # the PE duty cycle collapses there, HAM halves the clock,
                # and the tail runs cold).
                proj_kq(0)
                proj_v(0)
                proj_kq(1)
                for fc in range(2, DC):
                    for qc in range(2):
                        attn_pair(fc - 2, qc)
                    if fc == 4:
                        proj_v(1)
                    proj_kq(fc)
                for qc in range(2):                 # pair 6, both qc
                    attn_pair(DC - 2, qc)
                attn_pair(DC - 1, 0)                # pair 7 qc0
                # pair 7 qc1 emitted BEFORE out-proj t2=0: out-proj's fc0-6
                # matmuls are dependency-ready anyway (all qc0 heads done)
                # and fill endgame PE gaps, but this queue order keeps the
                # last odd-head shift DMA ahead of out-proj's output DMAs
                # on the sync queue
                attn_pair(DC - 1, 1)
                for ec in range(DC):
                    out_proj(ec, 0)
                for ec in range(DC):
                    out_proj(ec, 1)

    nc.compile()
    return nc


def _prep_in_maps(x, qkv_w, qkv_b, out_w, out_b):
    bf = ml_dtypes.bfloat16
    wqT = np.ascontiguousarray(qkv_w[0:D].T).astype(bf)
    wkT = np.ascontiguousarray(qkv_w[D:2 * D].T).astype(bf)
    wvT = np.ascontiguousarray(qkv_w[2 * D:3 * D].T).astype(bf)
    owT = np.ascontiguousarray(out_w.T).astype(bf)
    bq = np.ascontiguousarray(qkv_b[0:D].reshape(DC, 128).T).astype(np.float32)
    bk = np.ascontiguousarray(qkv_b[D:2 * D].reshape(DC, 128).T).astype(np.float32)
    ob_eff = out_b + out_w @ qkv_b[2 * D:3 * D]
    ob = np.ascontiguousarray(ob_eff.reshape(DC, 128).T).astype(np.float32)

    in_maps = []
    for i in range(N_CORES):
        b, h = i // 2, i % 2
        xb = x[b]
        xp = np.concatenate([xb[h * NQ:(h + 1) * NQ],
                             xb[(1 - h) * NQ:(2 - h) * NQ]], 0)
        xT = np.ascontiguousarray(xp.T).astype(bf)
        in_maps.append(dict(xT=xT, wqT=wqT, wkT=wkT, wvT=wvT, owT=owT,
                            bq=bq, bk=bk, ob=ob))
    return in_maps


def run(x, qkv_w, qkv_b, out_w, out_b, trace=False):
    if trace:
        _install_ntff_shim()
    if "nc" not in _CACHE:
        _CACHE["nc"] = build()
    nc = _CACHE["nc"]
    in_maps = _prep_in_maps(np.asarray(x, np.float32),
                            np.asarray(qkv_w, np.float32),
                            np.asarray(qkv_b, np.float32),
                            np.asarray(out_w, np.float32),
                            np.asarray(out_b, np.float32))
    res = run_bass_kernel_spmd(nc, in_maps, core_ids=list(range(N_CORES)),
                               trace=trace)
    out = np.empty((4, 2048, D), np.float32)
    for i in range(N_CORES):
        b, h = i // 2, i % 2
        out[b, h * NQ:(h + 1) * NQ] = res.results[i]["outT"].T
    return out, res


def kernel(**inputs):
    out, _ = run(**inputs)
    return out

